# revision 5
# baseline (speedup 1.0000x reference)
import sys

if '/opt/trn_rl_repo' not in sys.path:
    sys.path.insert(0, '/opt/trn_rl_repo')

import numpy as np

# Model dims (hardcoded from the problem spec)
B, C, N = 4, 512, 2048
NH, D = 8, 64          # heads, head dim
HID = 1024             # mlp hidden
NLOC = N // 2          # sequence half per core
CG = C // 128          # channel groups of 128
MT = N // 128          # m-tiles of 128 over full sequence
BN_EPS = 1e-5

_CACHE = {}


def _build_nc():
    import concourse.bacc as bacc
    import concourse.bass as bass
    import concourse.tile as tile
    import concourse.mybir as mybir
    from contextlib import ExitStack

    F32R, F32 = mybir.dt.float32r, mybir.dt.float32
    AF = mybir.ActivationFunctionType
    ALU = mybir.AluOpType

    nc = bacc.Bacc("TRN2")

    x_d = nc.dram_tensor("x", [C, N], F32R, kind="ExternalInput")
    wq_d = nc.dram_tensor("wqT", [C, C], F32R, kind="ExternalInput")
    wk_d = nc.dram_tensor("wkT", [C, C], F32R, kind="ExternalInput")
    wv_d = nc.dram_tensor("wvT", [C, C], F32R, kind="ExternalInput")
    wp_d = nc.dram_tensor("wpT", [C, C], F32R, kind="ExternalInput")
    w1_d = nc.dram_tensor("w1T", [C, HID], F32R, kind="ExternalInput")
    w2_d = nc.dram_tensor("w2T", [HID, C], F32R, kind="ExternalInput")
    bns_d = nc.dram_tensor("bns", [C, 1], F32, kind="ExternalInput")
    bnb_d = nc.dram_tensor("bnb", [C, 1], F32, kind="ExternalInput")
    ones_d = nc.dram_tensor("ones", [128, MT * NH], F32R, kind="ExternalInput")
    y_d = nc.dram_tensor("y", [C, NLOC], F32, kind="ExternalOutput")

    with tile.TileContext(nc) as tc, ExitStack() as ctx:
        pers = ctx.enter_context(tc.tile_pool(name="pers", bufs=1))

        xb = pers.tile([128, CG, N], F32R)
        for g in range(CG):
            nc.sync.dma_start(out=xb[:, g, :], in_=x_d[g * 128:(g + 1) * 128, :])
        wp_sb = pers.tile([128, CG, C], F32R)
        for c in range(CG):
            nc.sync.dma_start(out=wp_sb[:, c, :], in_=wp_d[c * 128:(c + 1) * 128, :])
        bns_sb = pers.tile([128, CG], F32)
        bnb_sb = pers.tile([128, CG], F32)
        for g in range(CG):
            nc.sync.dma_start(out=bns_sb[:, g:g + 1], in_=bns_d[g * 128:(g + 1) * 128, :])
            nc.sync.dma_start(out=bnb_sb[:, g:g + 1], in_=bnb_d[g * 128:(g + 1) * 128, :])
        attnout = pers.tile([128, CG, NLOC], F32R)
        y1 = pers.tile([128, CG, NLOC], F32R)

        with tc.tile_pool(name="attn_data", bufs=1) as ad:
            k_sb = ad.tile([128, CG, N], F32R)
            q_sb = ad.tile([128, CG, NLOC], F32R)
            vT = ad.tile([128, MT, NH * 65], F32R)
            # ones columns of vT (col 64 of each 65-wide head block)
            vT_ones = vT.rearrange("p m (h e) -> p (m h) e", e=65)[:, :, 64:65]
            nc.sync.dma_start(out=vT_ones,
                              in_=ones_d[:, :].rearrange("p (a b) -> p a b", b=1))

            # ---------------- Phase 1: qkv ----------------
            with tc.tile_pool(name="qkvw", bufs=1) as qw, \
                 tc.tile_pool(name="ps1", bufs=4, space="PSUM") as ps1:
                wq_sb = qw.tile([128, CG, C], F32R)
                wk_sb = qw.tile([128, CG, C], F32R)
                wv_sb = qw.tile([128, CG, C], F32R)
                for c in range(CG):
                    nc.sync.dma_start(out=wq_sb[:, c, :], in_=wq_d[c * 128:(c + 1) * 128, :])
                    nc.sync.dma_start(out=wk_sb[:, c, :], in_=wk_d[c * 128:(c + 1) * 128, :])
                    nc.sync.dma_start(out=wv_sb[:, c, :], in_=wv_d[c * 128:(c + 1) * 128, :])

                # k over the full sequence: k_sb[p, g, m] ; 4 out groups x 4 m chunks
                for g in range(CG):
                    for mc in range(N // 512):
                        ps = ps1.tile([128, 512], F32, tag="ps1")
                        for cc in range(CG):
                            nc.tensor.matmul(
                                ps,
                                wk_sb[:, cc, g * 128:(g + 1) * 128],
                                xb[:, cc, mc * 512:(mc + 1) * 512],
                                start=(cc == 0), stop=(cc == CG - 1))
                        if mc % 2 == 0:
                            nc.scalar.copy(out=k_sb[:, g, mc * 512:(mc + 1) * 512], in_=ps)
                        else:
                            nc.vector.tensor_copy(out=k_sb[:, g, mc * 512:(mc + 1) * 512], in_=ps)
                # q over local half
                for g in range(CG):
                    for qc in range(NLOC // 512):
                        ps = ps1.tile([128, 512], F32, tag="ps1")
                        for cc in range(CG):
                            nc.tensor.matmul(
                                ps,
                                wq_sb[:, cc, g * 128:(g + 1) * 128],
                                xb[:, cc, qc * 512:(qc + 1) * 512],
                                start=(cc == 0), stop=(cc == CG - 1))
                        if qc % 2 == 0:
                            nc.scalar.copy(out=q_sb[:, g, qc * 512:(qc + 1) * 512], in_=ps)
                        else:
                            nc.vector.tensor_copy(out=q_sb[:, g, qc * 512:(qc + 1) * 512], in_=ps)
                # vT over full sequence: per m-tile, all heads side by side
                for mt in range(MT):
                    ps = ps1.tile([128, 512], F32, tag="ps1")
                    for cc in range(CG):
                        nc.tensor.matmul(
                            ps,
                            xb[:, cc, mt * 128:(mt + 1) * 128],
                            wv_sb[:, cc, :],
                            start=(cc == 0), stop=(cc == CG - 1))
                    nc.vector.tensor_copy(
                        out=vT[:, mt, :].rearrange("p (h e) -> p h e", e=65)[:, :, 0:64],
                        in_=ps.rearrange("p (h e) -> p h e", e=64))

            # ---------------- Phase 2: attention ----------------
            with tc.tile_pool(name="eTp", bufs=3) as eTp, \
                 tc.tile_pool(name="nrm", bufs=2) as nrm, \
                 tc.tile_pool(name="nrm_dram", bufs=2, space="DRAM") as nrm_dram, \
                 tc.tile_pool(name="ps_sc", bufs=2, space="PSUM") as ps_sc, \
                 tc.tile_pool(name="ps_o", bufs=4, space="PSUM") as ps_o:
                for hp in range(NH // 2):
                    hA, hB = 2 * hp, 2 * hp + 1
                    for qc in range(NLOC // 512):
                        oA = ps_o.tile([65, 512], F32, tag="po")
                        oB = ps_o.tile([65, 512], F32, tag="po")
                        for mt in range(MT):
                            sc = ps_sc.tile([128, 1024], F32, tag="sc")
                            nc.tensor.matmul(
                                sc[:, 0:512],
                                k_sb[0:64, hp, mt * 128:(mt + 1) * 128],
                                q_sb[0:64, hp, qc * 512:(qc + 1) * 512],
                                start=True, stop=True, tile_position=(0, 0))
                            nc.tensor.matmul(
                                sc[:, 512:1024],
                                k_sb[64:128, hp, mt * 128:(mt + 1) * 128],
                                q_sb[64:128, hp, qc * 512:(qc + 1) * 512],
                                start=True, stop=True, tile_position=(64, 0))
                            eT = eTp.tile([128, 1024], F32R, tag="eT")
                            nc.scalar.activation(eT, sc, AF.Exp)
                            nc.tensor.matmul(
                                oA, vT[:, mt, hA * 65:(hA + 1) * 65], eT[:, 0:512],
                                start=(mt == 0), stop=(mt == MT - 1))
                            nc.tensor.matmul(
                                oB, vT[:, mt, hB * 65:(hB + 1) * 65], eT[:, 512:1024],
                                start=(mt == 0), stop=(mt == MT - 1))
                        # normalize by the ones-row sums and place into attnout
                        r = nrm.tile([1, 1024], F32, tag="r")
                        nc.vector.reciprocal(r[:, 0:512], oA[64:65, :])
                        nc.vector.reciprocal(r[:, 512:1024], oB[64:65, :])
                        r_dram = nrm_dram.tile([1, 1024], F32, tag="rd")
                        nc.gpsimd.dma_start(out=r_dram, in_=r[0:1, :])
                        bc = nrm.tile([64, 1024], F32, tag="bc")
                        rsrc = r_dram[0:1, :]
                        bsrc = bass.AP(tensor=rsrc.tensor, offset=rsrc.offset,
                                       ap=[[0, 64]] + [list(p) for p in rsrc.ap[1:]])
                        nc.gpsimd.dma_start(out=bc, in_=bsrc)
                        nc.vector.tensor_tensor(
                            out=attnout[0:64, hp, qc * 512:(qc + 1) * 512],
                            in0=oA[0:64, :], in1=bc[:, 0:512], op=ALU.mult)
                        tmpB = nrm.tile([64, 512], F32R, tag="tb")
                        nc.vector.tensor_tensor(
                            out=tmpB, in0=oB[0:64, :], in1=bc[:, 512:1024], op=ALU.mult)
                        nc.gpsimd.dma_start(
                            out=attnout[64:128, hp, qc * 512:(qc + 1) * 512], in_=tmpB)

        # ---------------- Phase 3-5: proj + BN1, MLP, BN2 ----------------
        with tc.tile_pool(name="mlpw", bufs=1) as mw, \
             tc.tile_pool(name="outp", bufs=2) as outp, \
             tc.tile_pool(name="ps_mm", bufs=4, space="PSUM") as ps_mm:
            w1_sb = mw.tile([128, CG, HID], F32R)
            for c in range(CG):
                nc.sync.dma_start(out=w1_sb[:, c, :], in_=w1_d[c * 128:(c + 1) * 128, :])
            w2_sb = mw.tile([128, HID // 128, C], F32R)
            for c in range(HID // 128):
                nc.sync.dma_start(out=w2_sb[:, c, :], in_=w2_d[c * 128:(c + 1) * 128, :])
            h_sb = mw.tile([128, HID // 128, NLOC], F32R)

            # proj + BN1 (+ residual x)
            for g in range(CG):
                ps = ps_mm.tile([128, NLOC], F32, tag="mm")
                for cc in range(CG):
                    for qc in range(NLOC // 512):
                        nc.tensor.matmul(
                            ps[:, qc * 512:(qc + 1) * 512],
                            wp_sb[:, cc, g * 128:(g + 1) * 128],
                            attnout[:, cc, qc * 512:(qc + 1) * 512],
                            start=(cc == 0), stop=(cc == CG - 1))
                nc.vector.tensor_tensor(out=y1[:, g, :], in0=ps, in1=xb[:, g, 0:NLOC],
                                        op=ALU.add)
                nc.vector.tensor_scalar(out=y1[:, g, :], in0=y1[:, g, :],
                                        scalar1=bns_sb[:, g:g + 1],
                                        scalar2=bnb_sb[:, g:g + 1],
                                        op0=ALU.mult, op1=ALU.add)
            # fc1 + relu
            for go in range(HID // 128):
                ps = ps_mm.tile([128, NLOC], F32, tag="mm")
                for cc in range(CG):
                    for qc in range(NLOC // 512):
                        nc.tensor.matmul(
                            ps[:, qc * 512:(qc + 1) * 512],
                            w1_sb[:, cc, go * 128:(go + 1) * 128],
                            y1[:, cc, qc * 512:(qc + 1) * 512],
                            start=(cc == 0), stop=(cc == CG - 1))
                nc.scalar.activation(h_sb[:, go, :], ps, AF.Relu)
            # fc2 + BN2 (+ residual y1)
            for g in range(CG):
                ps = ps_mm.tile([128, NLOC], F32, tag="mm")
                for hc in range(HID // 128):
                    for qc in range(NLOC // 512):
                        nc.tensor.matmul(
                            ps[:, qc * 512:(qc + 1) * 512],
                            w2_sb[:, hc, g * 128:(g + 1) * 128],
                            h_sb[:, hc, qc * 512:(qc + 1) * 512],
                            start=(hc == 0), stop=(hc == HID // 128 - 1))
                ob = outp.tile([128, NLOC], F32, tag="ob")
                nc.vector.tensor_tensor(out=ob, in0=ps, in1=y1[:, g, :], op=ALU.add)
                nc.vector.tensor_scalar(out=ob, in0=ob,
                                        scalar1=bns_sb[:, g:g + 1],
                                        scalar2=bnb_sb[:, g:g + 1],
                                        op0=ALU.mult, op1=ALU.add)
                nc.sync.dma_start(out=y_d[g * 128:(g + 1) * 128, :], in_=ob)

    nc.compile()
    return nc


def _host_prep(x, w_qkv, w_proj, w_fc1, w_fc2, gamma, beta, running_mean, running_var):
    x = np.asarray(x, np.float32)
    w_qkv = np.asarray(w_qkv, np.float32)
    bns = (np.asarray(gamma, np.float32)
           / np.sqrt(np.asarray(running_var, np.float32) + BN_EPS))
    bnb = np.asarray(beta, np.float32) - np.asarray(running_mean, np.float32) * bns
    wqT = np.ascontiguousarray(w_qkv[0:C].T) / np.float32(D ** 0.5)
    wkT = np.ascontiguousarray(w_qkv[C:2 * C].T)
    wvT = np.ascontiguousarray(w_qkv[2 * C:3 * C].T)
    wpT = np.ascontiguousarray(np.asarray(w_proj, np.float32).T)
    w1T = np.ascontiguousarray(np.asarray(w_fc1, np.float32).T)
    w2T = np.ascontiguousarray(np.asarray(w_fc2, np.float32).T)
    ones = np.ones((128, MT * NH), np.float32)
    common = dict(wqT=wqT, wkT=wkT, wvT=wvT, wpT=wpT, w1T=w1T, w2T=w2T,
                  bns=bns.reshape(C, 1).astype(np.float32),
                  bnb=bnb.reshape(C, 1).astype(np.float32), ones=ones)
    in_maps = []
    for core in range(8):
        b, s = core // 2, core % 2
        xb = np.ascontiguousarray(np.roll(x[b], -s * NLOC, axis=1))
        in_maps.append(dict(x=xb, **common))
    return x, in_maps


def kernel(x, w_qkv, w_proj, w_fc1, w_fc2, gamma, beta,
           running_mean, running_var, **_ignored):
    from concourse.bass_utils import run_bass_kernel_spmd
    if 'nc' not in _CACHE:
        _CACHE['nc'] = _build_nc()
    nc = _CACHE['nc']
    x, in_maps = _host_prep(x, w_qkv, w_proj, w_fc1, w_fc2, gamma, beta,
                            running_mean, running_var)
    res = run_bass_kernel_spmd(nc, in_maps, core_ids=list(range(8)))
    y = np.empty((B, C, N), np.float32)
    for core in range(8):
        b, s = core // 2, core % 2
        y[b][:, s * NLOC:(s + 1) * NLOC] = res.results[core]["y"]
    return y
